# revision 10
# baseline (speedup 1.0000x reference)
"""VQ codebook context-encoding kernel for 8 trn2 NeuronCores.

Math (factored): out[b,c] = (S1[b,c] - asum[b,:] @ cw[:,c]) / K
  S1[b,c]   = sum_n x[b,c,n]
  asum[b,k] = sum_n softmax_k(-scale[k]*dist[b,n,k])
  dist      = sqrt(f2[n] + c2[k] - 2*fc[n,k]);  fc = f @ cw.T, f2 = sum_c x^2

Key tricks vs the naive pipeline:
  * scale^2 folded into the matmul weights host-side: PSUM holds
    d2s = (scale_k * dist)^2 directly; sqrt via exp(0.5*ln d2s); the softmax
    logit is -sign_k*sqrt(d2s).  Codewords reordered host-side so positive
    scales come first -> the final Exp runs as two activation calls with
    scale=-1/+1 on the column halves (no per-k tensor multiply).
  * c2*s^2 accumulated into PSUM by one extra matmul against an all-ones
    stationary (hi/lo bf16 decomposition keeps ~f32 precision) — no
    elementwise add pass.
  * S1 rides on DVE tensor_scalar(mult 1, add 0) accum_out which keeps the
    4x_2p fast mode; x^2 via DVE tensor_tensor (2x mode) with a few chunks
    on ScalarE Square to balance ACT vs DVE.
  * One activation table (natural_log_exp_and_others) serves Ln/Exp/Square;
    the greedy inserter thrashes between two lesser tables, so post-compile
    the loads are rewritten into a single load of that table.
  * Software-pipelined emission (front/mid/back interleaved across samples)
    + half-chunk DMA + split groups on the last sample to shrink the tail.

Sharding: data-parallel over B (4 samples per core), codebook replicated.
"""

import numpy as np
import ml_dtypes
from contextlib import ExitStack

import concourse.bass as bass
import concourse.tile as tile
from concourse import bacc, mybir
from concourse.bass_utils import run_bass_kernel_spmd

B, C, HH, WW = 32, 256, 64, 64
N = HH * WW                # 4096
HALF = N // 2
K = 32
NCORES = 8
BPC = B // NCORES          # samples per core
CK = 2                     # 128-row chunks of C
NSUB = N // 128            # 32 n-subtiles per sample
DEFAULT_NPOS = 15          # positives in seed-0 scale; kernel() recomputes

F32 = mybir.dt.float32
BF16 = mybir.dt.bfloat16
AF = mybir.ActivationFunctionType
ALU = mybir.AluOpType

# natural_log_exp_and_others: contains ln, exp, square, identity, copy
ACT_TABLE_LN_EXP = 6

ACT_XSQ = {(0, 1), (1, 1)}           # (sample, chunk): x^2 on ScalarE
GRP = (1, 1, 1, 2)                   # ln/exp groups per sample (tail split)


def build_nc(npos=DEFAULT_NPOS):
    nc = bacc.Bacc("TRN2", target_bir_lowering=False, debug=False)

    x_d = nc.dram_tensor("x", [BPC, C, N], F32, kind="ExternalInput")
    rx_d = nc.dram_tensor("rx", [CK, 128, K], BF16, kind="ExternalInput")
    rq_d = nc.dram_tensor("rq", [128, K], BF16, kind="ExternalInput")
    crow_d = nc.dram_tensor("crow", [128, K], BF16, kind="ExternalInput")
    ones_d = nc.dram_tensor("ones", [128, 128], BF16, kind="ExternalInput")
    cwk_d = nc.dram_tensor("cwk", [K, C], F32, kind="ExternalInput")
    out_d = nc.dram_tensor("out", [128, BPC * CK], F32, kind="ExternalOutput")

    with tile.TileContext(nc) as tc, ExitStack() as ctx:
        consts = ctx.enter_context(tc.tile_pool(name="consts", bufs=1))
        xpool = ctx.enter_context(tc.tile_pool(name="xp", bufs=3))
        qpool = ctx.enter_context(tc.tile_pool(name="qp", bufs=3))
        work = ctx.enter_context(tc.tile_pool(name="wk", bufs=3))
        epool = ctx.enter_context(tc.tile_pool(name="ep", bufs=3))
        dpool = ctx.enter_context(tc.tile_pool(name="dp", bufs=2))
        dps_p = ctx.enter_context(
            tc.tile_pool(name="dps", bufs=2, space=bass.MemorySpace.PSUM))
        aps_p = ctx.enter_context(
            tc.tile_pool(name="aps", bufs=2, space=bass.MemorySpace.PSUM))
        fps_p = ctx.enter_context(
            tc.tile_pool(name="fps", bufs=2, space=bass.MemorySpace.PSUM))

        rx_sb = []
        for ci in range(CK):
            t = consts.tile([128, K], BF16, name=f"rx_sb{ci}")
            nc.sync.dma_start(t[:], rx_d[ci])
            rx_sb.append(t)
        rq_sb = consts.tile([128, K], BF16)
        nc.sync.dma_start(rq_sb[:], rq_d[:])
        crow_sb = consts.tile([128, K], BF16)
        nc.sync.dma_start(crow_sb[:], crow_d[:])
        ones_sb = consts.tile([128, 128], BF16)
        nc.sync.dma_start(ones_sb[:], ones_d[:])
        cwk_sb = consts.tile([K, C], F32)
        nc.sync.dma_start(cwk_sb[:], cwk_d[:])
        oall = consts.tile([128, BPC * CK], F32)

        st = {}

        def front(s):
            d = {}
            d["xbf"] = [xpool.tile([128, N], BF16, tag=f"xbf{ci}",
                                   name=f"xbf{ci}") for ci in range(CK)]
            for h in range(2):
                for ci in (1, 0):
                    nc.gpsimd.dma_start(
                        d["xbf"][ci][:, HALF * h:HALF * (h + 1)],
                        x_d[s, 128 * ci:128 * (ci + 1),
                            HALF * h:HALF * (h + 1)])
            d["xsq"] = [qpool.tile([128, N], BF16, tag=f"xsq{ci}",
                                   name=f"xsq{ci}") for ci in range(CK)]
            d["sq_pending"] = []
            for ci in range(CK):
                if (s, ci) in ACT_XSQ:
                    if GRP[s] > 1:
                        nc.scalar.activation(d["xsq"][ci][:, 0:HALF],
                                             d["xbf"][ci][:, 0:HALF],
                                             AF.Square)
                        d["sq_pending"].append(ci)
                    else:
                        for h in range(2):
                            sl = slice(HALF * h, HALF * (h + 1))
                            nc.scalar.activation(d["xsq"][ci][:, sl],
                                                 d["xbf"][ci][:, sl],
                                                 AF.Square)
                else:
                    for h in range(2):
                        sl = slice(HALF * h, HALF * (h + 1))
                        nc.vector.tensor_tensor(
                            d["xsq"][ci][:, sl], d["xbf"][ci][:, sl],
                            d["xbf"][ci][:, sl], ALU.mult)
            d["dps"] = dps_p.tile([128, NSUB * K], F32, tag="d", name="dps")
            d["u"] = work.tile([128, NSUB * K], F32, tag="u", name="u")
            d["ds"] = work.tile([128, NSUB * K], F32, tag="ds", name="ds")
            d["e"] = epool.tile([128, NSUB * K], BF16, tag="e", name="e")
            d["ssb"] = work.tile([128, NSUB], F32, tag="ssb", name="ssb")
            d["r"] = work.tile([128, NSUB], F32, tag="r", name="r")
            d["rbf"] = work.tile([128, NSUB], BF16, tag="rbf", name="rbf")
            st[s] = d

        def mid(s, g):
            d = st[s]
            spg = NSUB // GRP[s]
            jlo, jhi = g * spg, (g + 1) * spg
            xbf, xsq, dps = d["xbf"], d["xsq"], d["dps"]
            for j in range(jlo, jhi):
                nt = j * 128
                sl = dps[:, K * j:K * (j + 1)]
                nc.tensor.matmul(sl, ones_sb[:], crow_sb[:],
                                 start=True, stop=False)
                nc.tensor.matmul(sl, xbf[0][:, nt:nt + 128], rx_sb[0][:],
                                 start=False, stop=False)
                nc.tensor.matmul(sl, xbf[1][:, nt:nt + 128], rx_sb[1][:],
                                 start=False, stop=False)
                nc.tensor.matmul(sl, xsq[0][:, nt:nt + 128], rq_sb[:],
                                 start=False, stop=False)
                nc.tensor.matmul(sl, xsq[1][:, nt:nt + 128], rq_sb[:],
                                 start=False, stop=True)
            cl, ch = K * jlo, K * jhi
            nc.scalar.activation(d["u"][:, cl:ch], dps[:, cl:ch], AF.Ln)
            nc.scalar.activation(d["ds"][:, cl:ch], d["u"][:, cl:ch],
                                 AF.Exp, scale=0.5)
            ds3 = d["ds"][:].rearrange("p (j k) -> p j k", k=K)
            e3 = d["e"][:].rearrange("p (j k) -> p j k", k=K)
            if npos > 0:
                nc.scalar.activation(e3[:, jlo:jhi, 0:npos],
                                     ds3[:, jlo:jhi, 0:npos],
                                     AF.Exp, scale=-1.0)
            if npos < K:
                nc.scalar.activation(e3[:, jlo:jhi, npos:K],
                                     ds3[:, jlo:jhi, npos:K],
                                     AF.Exp, scale=1.0)

        def backg(s, g):
            d = st[s]
            spg = NSUB // GRP[s]
            jlo, jhi = g * spg, (g + 1) * spg
            e3 = d["e"][:].rearrange("p (j k) -> p j k", k=K)
            nc.vector.tensor_reduce(
                d["ssb"][:, jlo:jhi], e3[:, jlo:jhi, :],
                axis=mybir.AxisListType.X, op=ALU.add)
            nc.vector.reciprocal(d["r"][:, jlo:jhi], d["ssb"][:, jlo:jhi])
            nc.vector.tensor_copy(d["rbf"][:, jlo:jhi], d["r"][:, jlo:jhi])
            if g == 0:
                d["asum"] = aps_p.tile([K, 1], F32, tag="asum", name="asum")
            for j in range(jlo, jhi):
                nc.tensor.matmul(d["asum"][:], d["e"][:, K * j:K * (j + 1)],
                                 d["rbf"][:, j:j + 1],
                                 start=(j == 0), stop=(j == NSUB - 1),
                                 skip_group_check=True)

        def backf(s):
            d = st[s]
            asum_sb = work.tile([K, 1], F32, tag="asum_sb", name="asum_sb")
            nc.vector.tensor_copy(asum_sb[:], d["asum"][:])
            for ci in range(CK):
                fps = fps_p.tile([128, 1], F32, tag="fin", name="fin")
                nc.tensor.matmul(fps[:], cwk_sb[:, 128 * ci:128 * (ci + 1)],
                                 asum_sb[:], start=True, stop=True)
                # out = s1/K - (asum@cw)/K  (cwk pre-scaled by 1/K on host)
                nc.vector.scalar_tensor_tensor(
                    oall[:, s * CK + ci:s * CK + ci + 1], d["s1"][ci][:],
                    1.0 / K, fps[:], ALU.mult, ALU.subtract)

        def mids(s):
            for g in range(GRP[s]):
                mid(s, g)
                # second square half after the first group's activations
                for ci in st[s].pop("sq_pending", []) if g == 0 else []:
                    nc.scalar.activation(
                        st[s]["xsq"][ci][:, HALF:N],
                        st[s]["xbf"][ci][:, HALF:N], AF.Square)

        def backs(s):
            d = st[s]
            # S1[c] = sum_n x — tensor_scalar accum keeps the 4x DVE mode;
            # placed here so it runs on DVE while ACT computes this sample's e
            d["s1"] = [work.tile([128, 1], F32, tag=f"s1{ci}",
                                 name=f"s1{ci}") for ci in range(CK)]
            for ci in range(CK):
                dump = dpool.tile([128, N], BF16, tag="dump", name="dump")
                nc.vector.tensor_scalar(
                    dump[:], d["xbf"][ci][:], 1.0, 0.0, ALU.mult, ALU.add,
                    accum_out=d["s1"][ci][:])
            for g in range(GRP[s]):
                backg(s, g)
            backf(s)

        front(0)
        front(1)
        mids(0)
        front(2)
        mids(1)
        backs(0)
        front(3)
        mids(2)
        backs(1)
        mids(3)
        backs(2)
        nc.sync.dma_start(out_d[:, 0:3 * CK], oall[:, 0:3 * CK])
        backs(3)
        nc.sync.dma_start(out_d[:, 3 * CK:BPC * CK], oall[:, 3 * CK:BPC * CK])
    nc.compile()

    # The greedy table-load inserter alternates natural_log <-> exp_and_others;
    # one load of natural_log_exp_and_others serves every Ln/Exp/Square here.
    for blk in nc.m.functions[0].blocks:
        insns = blk.instructions
        idxs = [i for i, it in enumerate(insns)
                if isinstance(it, mybir.InstLoadActFuncSet)]
        if not idxs:
            continue
        insns[idxs[0]].act_func_set_id = ACT_TABLE_LN_EXP
        for i in reversed(idxs[1:]):
            del insns[i]
    return nc


_NC = None
_NC_NPOS = None


def _get_nc(npos=None):
    global _NC, _NC_NPOS
    want = DEFAULT_NPOS if npos is None else npos
    if _NC is None or _NC_NPOS != want:
        _NC = build_nc(want)
        _NC_NPOS = want
    return _NC


def kernel(x, codewords, scale):
    x = np.ascontiguousarray(np.asarray(x, dtype=np.float32)).reshape(B, C, N)
    cw = np.asarray(codewords, dtype=np.float32)
    sc = np.asarray(scale, dtype=np.float32)

    # reorder codewords so positive scales come first (sign-split Exp)
    order = np.argsort(sc <= 0, kind="stable")
    cwo = cw[order].astype(np.float64)
    sco = sc[order].astype(np.float64)
    npos = int((sco > 0).sum())
    s2 = sco ** 2

    rx = (-2.0 * cwo.T * s2[None, :]).astype(ml_dtypes.bfloat16)
    rx = np.ascontiguousarray(rx.reshape(CK, 128, K))
    rq = np.tile(s2[None, :], (128, 1)).astype(ml_dtypes.bfloat16)
    c2s2 = (cwo ** 2).sum(axis=1) * s2
    hi = c2s2.astype(ml_dtypes.bfloat16)
    lo = (c2s2 - hi.astype(np.float64)).astype(ml_dtypes.bfloat16)
    crow = np.zeros((128, K), dtype=ml_dtypes.bfloat16)
    crow[0] = hi
    crow[1] = lo
    ones = np.ones((128, 128), dtype=ml_dtypes.bfloat16)
    cwk = (cwo / K).astype(np.float32)

    in_maps = []
    for core in range(NCORES):
        in_maps.append({
            "x": x[core * BPC:(core + 1) * BPC],
            "rx": rx, "rq": rq, "crow": crow, "ones": ones, "cwk": cwk,
        })

    res = run_bass_kernel_spmd(_get_nc(npos), in_maps,
                               core_ids=list(range(NCORES)))
    out = np.empty((B, C), dtype=np.float32)
    for core in range(NCORES):
        o = res.results[core]["out"]                    # [128, BPC*CK]
        for s in range(BPC):
            for ci in range(CK):
                out[core * BPC + s, 128 * ci:128 * (ci + 1)] = o[:, s * CK + ci]
    return out


# revision 11
# speedup vs baseline: 1.0098x; 1.0098x over previous
"""VQ codebook context-encoding kernel for 8 trn2 NeuronCores.

Math (factored): out[b,c] = (S1[b,c] - asum[b,:] @ cw[:,c]) / K
  S1[b,c]   = sum_n x[b,c,n]
  asum[b,k] = sum_n softmax_k(-scale[k]*dist[b,n,k])
  dist      = sqrt(f2[n] + c2[k] - 2*fc[n,k]);  fc = f @ cw.T, f2 = sum_c x^2

Key tricks vs the naive pipeline:
  * scale^2 folded into the matmul weights host-side: PSUM holds
    d2s = (scale_k * dist)^2 directly; sqrt via exp(0.5*ln d2s); the softmax
    logit is -sign_k*sqrt(d2s).  Codewords reordered host-side so positive
    scales come first -> the final Exp runs as two activation calls with
    scale=-1/+1 on the column halves (no per-k tensor multiply).
  * c2*s^2 accumulated into PSUM by one extra matmul against an all-ones
    stationary (hi/lo bf16 decomposition keeps ~f32 precision) — no
    elementwise add pass.
  * S1 rides on DVE tensor_scalar(mult 1, add 0) accum_out which keeps the
    4x_2p fast mode; x^2 via DVE tensor_tensor (2x mode) with a few chunks
    on ScalarE Square to balance ACT vs DVE.
  * One activation table (natural_log_exp_and_others) serves Ln/Exp/Square;
    the greedy inserter thrashes between two lesser tables, so post-compile
    the loads are rewritten into a single load of that table.
  * Software-pipelined emission (front/mid/back interleaved across samples)
    + half-chunk DMA + split groups on the last sample to shrink the tail.

Sharding: data-parallel over B (4 samples per core), codebook replicated.
"""

import numpy as np
import ml_dtypes
from contextlib import ExitStack

import concourse.bass as bass
import concourse.tile as tile
from concourse import bacc, mybir
from concourse.bass_utils import run_bass_kernel_spmd

B, C, HH, WW = 32, 256, 64, 64
N = HH * WW                # 4096
HALF = N // 2
K = 32
NCORES = 8
BPC = B // NCORES          # samples per core
CK = 2                     # 128-row chunks of C
NSUB = N // 128            # 32 n-subtiles per sample
DEFAULT_NPOS = 15          # positives in seed-0 scale; kernel() recomputes

F32 = mybir.dt.float32
BF16 = mybir.dt.bfloat16
AF = mybir.ActivationFunctionType
ALU = mybir.AluOpType

# natural_log_exp_and_others: contains ln, exp, square, identity, copy
ACT_TABLE_LN_EXP = 6

ACT_XSQ = {(0, 1), (1, 1), (2, 1)}   # (sample, chunk): x^2 on ScalarE
GRP = (2, 1, 1, 2)                   # ln/exp groups per sample (tail split)


def build_nc(npos=DEFAULT_NPOS):
    nc = bacc.Bacc("TRN2", target_bir_lowering=False, debug=False)

    x_d = nc.dram_tensor("x", [BPC, C, N], F32, kind="ExternalInput")
    rx_d = nc.dram_tensor("rx", [CK, 128, K], BF16, kind="ExternalInput")
    rq_d = nc.dram_tensor("rq", [128, K], BF16, kind="ExternalInput")
    crow_d = nc.dram_tensor("crow", [128, K], BF16, kind="ExternalInput")
    ones_d = nc.dram_tensor("ones", [128, 128], BF16, kind="ExternalInput")
    cwk_d = nc.dram_tensor("cwk", [K, C], F32, kind="ExternalInput")
    out_d = nc.dram_tensor("out", [128, BPC * CK], F32, kind="ExternalOutput")

    with tile.TileContext(nc) as tc, ExitStack() as ctx:
        consts = ctx.enter_context(tc.tile_pool(name="consts", bufs=1))
        xpool = ctx.enter_context(tc.tile_pool(name="xp", bufs=3))
        qpool = ctx.enter_context(tc.tile_pool(name="qp", bufs=3))
        work = ctx.enter_context(tc.tile_pool(name="wk", bufs=3))
        epool = ctx.enter_context(tc.tile_pool(name="ep", bufs=3))
        dpool = ctx.enter_context(tc.tile_pool(name="dp", bufs=2))
        dps_p = ctx.enter_context(
            tc.tile_pool(name="dps", bufs=2, space=bass.MemorySpace.PSUM))
        aps_p = ctx.enter_context(
            tc.tile_pool(name="aps", bufs=2, space=bass.MemorySpace.PSUM))
        fps_p = ctx.enter_context(
            tc.tile_pool(name="fps", bufs=2, space=bass.MemorySpace.PSUM))

        rx_sb = []
        for ci in range(CK):
            t = consts.tile([128, K], BF16, name=f"rx_sb{ci}")
            nc.sync.dma_start(t[:], rx_d[ci])
            rx_sb.append(t)
        rq_sb = consts.tile([128, K], BF16)
        nc.sync.dma_start(rq_sb[:], rq_d[:])
        crow_sb = consts.tile([128, K], BF16)
        nc.sync.dma_start(crow_sb[:], crow_d[:])
        ones_sb = consts.tile([128, 128], BF16)
        nc.sync.dma_start(ones_sb[:], ones_d[:])
        cwk_sb = consts.tile([K, C], F32)
        nc.sync.dma_start(cwk_sb[:], cwk_d[:])
        oall = consts.tile([128, BPC * CK], F32)

        st = {}

        def front(s):
            d = {}
            d["xbf"] = [xpool.tile([128, N], BF16, tag=f"xbf{ci}",
                                   name=f"xbf{ci}") for ci in range(CK)]
            for h in range(2):
                for ci in (1, 0):
                    nc.gpsimd.dma_start(
                        d["xbf"][ci][:, HALF * h:HALF * (h + 1)],
                        x_d[s, 128 * ci:128 * (ci + 1),
                            HALF * h:HALF * (h + 1)])
            d["xsq"] = [qpool.tile([128, N], BF16, tag=f"xsq{ci}",
                                   name=f"xsq{ci}") for ci in range(CK)]
            d["sq_pending"] = []
            for ci in range(CK):
                if (s, ci) in ACT_XSQ:
                    if GRP[s] > 1:
                        nc.scalar.activation(d["xsq"][ci][:, 0:HALF],
                                             d["xbf"][ci][:, 0:HALF],
                                             AF.Square)
                        d["sq_pending"].append(ci)
                    else:
                        for h in range(2):
                            sl = slice(HALF * h, HALF * (h + 1))
                            nc.scalar.activation(d["xsq"][ci][:, sl],
                                                 d["xbf"][ci][:, sl],
                                                 AF.Square)
                else:
                    for h in range(2):
                        sl = slice(HALF * h, HALF * (h + 1))
                        nc.vector.tensor_tensor(
                            d["xsq"][ci][:, sl], d["xbf"][ci][:, sl],
                            d["xbf"][ci][:, sl], ALU.mult)
            d["dps"] = dps_p.tile([128, NSUB * K], F32, tag="d", name="dps")
            d["u"] = work.tile([128, NSUB * K], F32, tag="u", name="u")
            d["ds"] = work.tile([128, NSUB * K], F32, tag="ds", name="ds")
            d["e"] = epool.tile([128, NSUB * K], BF16, tag="e", name="e")
            d["ssb"] = work.tile([128, NSUB], F32, tag="ssb", name="ssb")
            d["r"] = work.tile([128, NSUB], F32, tag="r", name="r")
            d["rbf"] = work.tile([128, NSUB], BF16, tag="rbf", name="rbf")
            st[s] = d

        def mid(s, g):
            d = st[s]
            spg = NSUB // GRP[s]
            jlo, jhi = g * spg, (g + 1) * spg
            xbf, xsq, dps = d["xbf"], d["xsq"], d["dps"]
            for j in range(jlo, jhi):
                nt = j * 128
                sl = dps[:, K * j:K * (j + 1)]
                nc.tensor.matmul(sl, ones_sb[:], crow_sb[:],
                                 start=True, stop=False)
                nc.tensor.matmul(sl, xbf[0][:, nt:nt + 128], rx_sb[0][:],
                                 start=False, stop=False)
                nc.tensor.matmul(sl, xbf[1][:, nt:nt + 128], rx_sb[1][:],
                                 start=False, stop=False)
                nc.tensor.matmul(sl, xsq[0][:, nt:nt + 128], rq_sb[:],
                                 start=False, stop=False)
                nc.tensor.matmul(sl, xsq[1][:, nt:nt + 128], rq_sb[:],
                                 start=False, stop=True)
            cl, ch = K * jlo, K * jhi
            nc.scalar.activation(d["u"][:, cl:ch], dps[:, cl:ch], AF.Ln)
            nc.scalar.activation(d["ds"][:, cl:ch], d["u"][:, cl:ch],
                                 AF.Exp, scale=0.5)
            ds3 = d["ds"][:].rearrange("p (j k) -> p j k", k=K)
            e3 = d["e"][:].rearrange("p (j k) -> p j k", k=K)
            if npos > 0:
                nc.scalar.activation(e3[:, jlo:jhi, 0:npos],
                                     ds3[:, jlo:jhi, 0:npos],
                                     AF.Exp, scale=-1.0)
            if npos < K:
                nc.scalar.activation(e3[:, jlo:jhi, npos:K],
                                     ds3[:, jlo:jhi, npos:K],
                                     AF.Exp, scale=1.0)

        def backg(s, g):
            d = st[s]
            spg = NSUB // GRP[s]
            jlo, jhi = g * spg, (g + 1) * spg
            e3 = d["e"][:].rearrange("p (j k) -> p j k", k=K)
            nc.vector.tensor_reduce(
                d["ssb"][:, jlo:jhi], e3[:, jlo:jhi, :],
                axis=mybir.AxisListType.X, op=ALU.add)
            nc.vector.reciprocal(d["r"][:, jlo:jhi], d["ssb"][:, jlo:jhi])
            nc.vector.tensor_copy(d["rbf"][:, jlo:jhi], d["r"][:, jlo:jhi])
            if g == 0:
                d["asum"] = aps_p.tile([K, 1], F32, tag="asum", name="asum")
            for j in range(jlo, jhi):
                nc.tensor.matmul(d["asum"][:], d["e"][:, K * j:K * (j + 1)],
                                 d["rbf"][:, j:j + 1],
                                 start=(j == 0), stop=(j == NSUB - 1),
                                 skip_group_check=True)

        def backf(s):
            d = st[s]
            asum_sb = work.tile([K, 1], F32, tag="asum_sb", name="asum_sb")
            nc.vector.tensor_copy(asum_sb[:], d["asum"][:])
            for ci in range(CK):
                fps = fps_p.tile([128, 1], F32, tag="fin", name="fin")
                nc.tensor.matmul(fps[:], cwk_sb[:, 128 * ci:128 * (ci + 1)],
                                 asum_sb[:], start=True, stop=True)
                # out = s1/K - (asum@cw)/K  (cwk pre-scaled by 1/K on host)
                nc.vector.scalar_tensor_tensor(
                    oall[:, s * CK + ci:s * CK + ci + 1], d["s1"][ci][:],
                    1.0 / K, fps[:], ALU.mult, ALU.subtract)

        def mids(s):
            for g in range(GRP[s]):
                mid(s, g)
                # second square half after the first group's activations
                for ci in st[s].pop("sq_pending", []) if g == 0 else []:
                    nc.scalar.activation(
                        st[s]["xsq"][ci][:, HALF:N],
                        st[s]["xbf"][ci][:, HALF:N], AF.Square)

        def backs(s):
            d = st[s]
            # S1[c] = sum_n x — tensor_scalar accum keeps the 4x DVE mode;
            # placed here so it runs on DVE while ACT computes this sample's e
            d["s1"] = [work.tile([128, 1], F32, tag=f"s1{ci}",
                                 name=f"s1{ci}") for ci in range(CK)]
            for ci in range(CK):
                dump = dpool.tile([128, N], BF16, tag="dump", name="dump")
                nc.vector.tensor_scalar(
                    dump[:], d["xbf"][ci][:], 1.0, 0.0, ALU.mult, ALU.add,
                    accum_out=d["s1"][ci][:])
            for g in range(GRP[s]):
                backg(s, g)
            backf(s)

        front(0)
        front(1)
        mids(0)
        front(2)
        mids(1)
        backs(0)
        front(3)
        mids(2)
        backs(1)
        mids(3)
        backs(2)
        nc.sync.dma_start(out_d[:, 0:3 * CK], oall[:, 0:3 * CK])
        backs(3)
        nc.sync.dma_start(out_d[:, 3 * CK:BPC * CK], oall[:, 3 * CK:BPC * CK])
    nc.compile()

    # The greedy table-load inserter alternates natural_log <-> exp_and_others;
    # one load of natural_log_exp_and_others serves every Ln/Exp/Square here.
    for blk in nc.m.functions[0].blocks:
        insns = blk.instructions
        idxs = [i for i, it in enumerate(insns)
                if isinstance(it, mybir.InstLoadActFuncSet)]
        if not idxs:
            continue
        insns[idxs[0]].act_func_set_id = ACT_TABLE_LN_EXP
        for i in reversed(idxs[1:]):
            del insns[i]
    return nc


_NC = None
_NC_NPOS = None


def _get_nc(npos=None):
    global _NC, _NC_NPOS
    want = DEFAULT_NPOS if npos is None else npos
    if _NC is None or _NC_NPOS != want:
        _NC = build_nc(want)
        _NC_NPOS = want
    return _NC


def kernel(x, codewords, scale):
    x = np.ascontiguousarray(np.asarray(x, dtype=np.float32)).reshape(B, C, N)
    cw = np.asarray(codewords, dtype=np.float32)
    sc = np.asarray(scale, dtype=np.float32)

    # reorder codewords so positive scales come first (sign-split Exp)
    order = np.argsort(sc <= 0, kind="stable")
    cwo = cw[order].astype(np.float64)
    sco = sc[order].astype(np.float64)
    npos = int((sco > 0).sum())
    s2 = sco ** 2

    rx = (-2.0 * cwo.T * s2[None, :]).astype(ml_dtypes.bfloat16)
    rx = np.ascontiguousarray(rx.reshape(CK, 128, K))
    rq = np.tile(s2[None, :], (128, 1)).astype(ml_dtypes.bfloat16)
    c2s2 = (cwo ** 2).sum(axis=1) * s2
    hi = c2s2.astype(ml_dtypes.bfloat16)
    lo = (c2s2 - hi.astype(np.float64)).astype(ml_dtypes.bfloat16)
    crow = np.zeros((128, K), dtype=ml_dtypes.bfloat16)
    crow[0] = hi
    crow[1] = lo
    ones = np.ones((128, 128), dtype=ml_dtypes.bfloat16)
    cwk = (cwo / K).astype(np.float32)

    in_maps = []
    for core in range(NCORES):
        in_maps.append({
            "x": x[core * BPC:(core + 1) * BPC],
            "rx": rx, "rq": rq, "crow": crow, "ones": ones, "cwk": cwk,
        })

    res = run_bass_kernel_spmd(_get_nc(npos), in_maps,
                               core_ids=list(range(NCORES)))
    out = np.empty((B, C), dtype=np.float32)
    for core in range(NCORES):
        o = res.results[core]["out"]                    # [128, BPC*CK]
        for s in range(BPC):
            for ci in range(CK):
                out[core * BPC + s, 128 * ci:128 * (ci + 1)] = o[:, s * CK + ci]
    return out


# revision 13
# speedup vs baseline: 1.0410x; 1.0309x over previous
"""VQ codebook context-encoding kernel for 8 trn2 NeuronCores.

Math (factored): out[b,c] = (S1[b,c] - asum[b,:] @ cw[:,c]) / K
  S1[b,c]   = sum_n x[b,c,n]
  asum[b,k] = sum_n softmax_k(-scale[k]*dist[b,n,k])
  dist      = sqrt(f2[n] + c2[k] - 2*fc[n,k]);  fc = f @ cw.T, f2 = sum_c x^2

Key tricks vs the naive pipeline:
  * scale^2 folded into the matmul weights host-side: PSUM holds
    d2s = (scale_k * dist)^2 directly; sqrt via exp(0.5*ln d2s); the softmax
    logit is -sign_k*sqrt(d2s).  Codewords reordered host-side so positive
    scales come first -> the final Exp runs as two activation calls with
    scale=-1/+1 on the column halves (no per-k tensor multiply).
  * c2*s^2 accumulated into PSUM by one extra matmul against an all-ones
    stationary (hi/lo bf16 decomposition keeps ~f32 precision) — no
    elementwise add pass.
  * S1 rides on DVE tensor_scalar(mult 1, add 0) accum_out which keeps the
    4x_2p fast mode; x^2 via DVE tensor_tensor (2x mode) with a few chunks
    on ScalarE Square to balance ACT vs DVE.
  * One activation table (natural_log_exp_and_others) serves Ln/Exp/Square;
    the greedy inserter thrashes between two lesser tables, so post-compile
    the loads are rewritten into a single load of that table.
  * Software-pipelined emission (front/mid/back interleaved across samples)
    + half-chunk DMA + split groups on the last sample to shrink the tail.

Sharding: data-parallel over B (4 samples per core), codebook replicated.
"""

import numpy as np
import ml_dtypes
from contextlib import ExitStack

import concourse.bass as bass
import concourse.tile as tile
from concourse import bacc, mybir
from concourse.bass_utils import run_bass_kernel_spmd

B, C, HH, WW = 32, 256, 64, 64
N = HH * WW                # 4096
HALF = N // 2
K = 32
NCORES = 8
BPC = B // NCORES          # samples per core
CK = 2                     # 128-row chunks of C
NSUB = N // 128            # 32 n-subtiles per sample
DEFAULT_NPOS = 15          # positives in seed-0 scale; kernel() recomputes

F32 = mybir.dt.float32
BF16 = mybir.dt.bfloat16
AF = mybir.ActivationFunctionType
ALU = mybir.AluOpType

# natural_log_exp_and_others: contains ln, exp, square, identity, copy
ACT_TABLE_LN_EXP = 6

ACT_XSQ = {(0, 1), (1, 1), (2, 1)}   # (sample, chunk): x^2 on ScalarE
GRP = (1, 1, 1, 2)                   # ln/exp groups per sample (tail split)


def build_nc(npos=DEFAULT_NPOS):
    nc = bacc.Bacc("TRN2", target_bir_lowering=False, debug=False)

    x_d = nc.dram_tensor("x", [BPC, C, N], F32, kind="ExternalInput")
    rx_d = nc.dram_tensor("rx", [CK, 128, K], BF16, kind="ExternalInput")
    rq_d = nc.dram_tensor("rq", [128, K], BF16, kind="ExternalInput")
    crow_d = nc.dram_tensor("crow", [128, K], BF16, kind="ExternalInput")
    ones_d = nc.dram_tensor("ones", [128, 128], BF16, kind="ExternalInput")
    cwk_d = nc.dram_tensor("cwk", [K, C], F32, kind="ExternalInput")
    out_d = nc.dram_tensor("out", [128, BPC * CK], F32, kind="ExternalOutput")

    with tile.TileContext(nc) as tc, ExitStack() as ctx:
        consts = ctx.enter_context(tc.tile_pool(name="consts", bufs=1))
        xpool = ctx.enter_context(tc.tile_pool(name="xp", bufs=4))
        qpool = ctx.enter_context(tc.tile_pool(name="qp", bufs=3))
        work = ctx.enter_context(tc.tile_pool(name="wk", bufs=3))
        epool = ctx.enter_context(tc.tile_pool(name="ep", bufs=3))
        dpool = ctx.enter_context(tc.tile_pool(name="dp", bufs=2))
        dps_p = ctx.enter_context(
            tc.tile_pool(name="dps", bufs=2, space=bass.MemorySpace.PSUM))
        aps_p = ctx.enter_context(
            tc.tile_pool(name="aps", bufs=2, space=bass.MemorySpace.PSUM))
        fps_p = ctx.enter_context(
            tc.tile_pool(name="fps", bufs=2, space=bass.MemorySpace.PSUM))

        rx_sb = []
        for ci in range(CK):
            t = consts.tile([128, K], BF16, name=f"rx_sb{ci}")
            nc.sync.dma_start(t[:], rx_d[ci])
            rx_sb.append(t)
        rq_sb = consts.tile([128, K], BF16)
        nc.sync.dma_start(rq_sb[:], rq_d[:])
        crow_sb = consts.tile([128, K], BF16)
        nc.sync.dma_start(crow_sb[:], crow_d[:])
        ones_sb = consts.tile([128, 128], BF16)
        nc.sync.dma_start(ones_sb[:], ones_d[:])
        cwk_sb = consts.tile([K, C], F32)
        nc.sync.dma_start(cwk_sb[:], cwk_d[:])
        oall = consts.tile([128, BPC * CK], F32)

        st = {}

        def front(s):
            d = {}
            d["xbf"] = [xpool.tile([128, N], BF16, tag=f"xbf{ci}",
                                   name=f"xbf{ci}") for ci in range(CK)]
            for h in range(2):
                for ci in (1, 0):
                    nc.gpsimd.dma_start(
                        d["xbf"][ci][:, HALF * h:HALF * (h + 1)],
                        x_d[s, 128 * ci:128 * (ci + 1),
                            HALF * h:HALF * (h + 1)])
            d["xsq"] = [qpool.tile([128, N], BF16, tag=f"xsq{ci}",
                                   name=f"xsq{ci}") for ci in range(CK)]
            d["sq_pending"] = []
            for ci in range(CK):
                if (s, ci) in ACT_XSQ:
                    if GRP[s] > 1:
                        nc.scalar.activation(d["xsq"][ci][:, 0:HALF],
                                             d["xbf"][ci][:, 0:HALF],
                                             AF.Square)
                        d["sq_pending"].append(ci)
                    else:
                        for h in range(2):
                            sl = slice(HALF * h, HALF * (h + 1))
                            nc.scalar.activation(d["xsq"][ci][:, sl],
                                                 d["xbf"][ci][:, sl],
                                                 AF.Square)
                else:
                    for h in range(2):
                        sl = slice(HALF * h, HALF * (h + 1))
                        nc.vector.tensor_tensor(
                            d["xsq"][ci][:, sl], d["xbf"][ci][:, sl],
                            d["xbf"][ci][:, sl], ALU.mult)
            d["dps"] = dps_p.tile([128, NSUB * K], F32, tag="d", name="dps")
            d["u"] = work.tile([128, NSUB * K], F32, tag="u", name="u")
            d["ds"] = work.tile([128, NSUB * K], F32, tag="ds", name="ds")
            d["e"] = epool.tile([128, NSUB * K], BF16, tag="e", name="e")
            d["ssb"] = work.tile([128, NSUB], F32, tag="ssb", name="ssb")
            d["r"] = work.tile([128, NSUB], F32, tag="r", name="r")
            d["rbf"] = work.tile([128, NSUB], BF16, tag="rbf", name="rbf")
            st[s] = d

        def mid(s, g):
            d = st[s]
            spg = NSUB // GRP[s]
            jlo, jhi = g * spg, (g + 1) * spg
            xbf, xsq, dps = d["xbf"], d["xsq"], d["dps"]
            for j in range(jlo, jhi):
                nt = j * 128
                sl = dps[:, K * j:K * (j + 1)]
                nc.tensor.matmul(sl, ones_sb[:], crow_sb[:],
                                 start=True, stop=False)
                nc.tensor.matmul(sl, xbf[0][:, nt:nt + 128], rx_sb[0][:],
                                 start=False, stop=False)
                nc.tensor.matmul(sl, xbf[1][:, nt:nt + 128], rx_sb[1][:],
                                 start=False, stop=False)
                nc.tensor.matmul(sl, xsq[0][:, nt:nt + 128], rq_sb[:],
                                 start=False, stop=False)
                nc.tensor.matmul(sl, xsq[1][:, nt:nt + 128], rq_sb[:],
                                 start=False, stop=True)
            cl, ch = K * jlo, K * jhi
            nc.scalar.activation(d["u"][:, cl:ch], dps[:, cl:ch], AF.Ln)
            nc.scalar.activation(d["ds"][:, cl:ch], d["u"][:, cl:ch],
                                 AF.Exp, scale=0.5)
            ds3 = d["ds"][:].rearrange("p (j k) -> p j k", k=K)
            e3 = d["e"][:].rearrange("p (j k) -> p j k", k=K)
            if npos > 0:
                nc.scalar.activation(e3[:, jlo:jhi, 0:npos],
                                     ds3[:, jlo:jhi, 0:npos],
                                     AF.Exp, scale=-1.0)
            if npos < K:
                nc.scalar.activation(e3[:, jlo:jhi, npos:K],
                                     ds3[:, jlo:jhi, npos:K],
                                     AF.Exp, scale=1.0)

        def backg(s, g):
            d = st[s]
            spg = NSUB // GRP[s]
            jlo, jhi = g * spg, (g + 1) * spg
            e3 = d["e"][:].rearrange("p (j k) -> p j k", k=K)
            nc.vector.tensor_reduce(
                d["ssb"][:, jlo:jhi], e3[:, jlo:jhi, :],
                axis=mybir.AxisListType.X, op=ALU.add)
            nc.vector.reciprocal(d["r"][:, jlo:jhi], d["ssb"][:, jlo:jhi])
            nc.vector.tensor_copy(d["rbf"][:, jlo:jhi], d["r"][:, jlo:jhi])
            if g == 0:
                d["asum"] = aps_p.tile([K, 1], F32, tag="asum", name="asum")
            for j in range(jlo, jhi):
                nc.tensor.matmul(d["asum"][:], d["e"][:, K * j:K * (j + 1)],
                                 d["rbf"][:, j:j + 1],
                                 start=(j == 0), stop=(j == NSUB - 1),
                                 skip_group_check=True)

        def backf(s):
            d = st[s]
            asum_sb = work.tile([K, 1], F32, tag="asum_sb", name="asum_sb")
            nc.vector.tensor_copy(asum_sb[:], d["asum"][:])
            for ci in range(CK):
                fps = fps_p.tile([128, 1], F32, tag="fin", name="fin")
                nc.tensor.matmul(fps[:], cwk_sb[:, 128 * ci:128 * (ci + 1)],
                                 asum_sb[:], start=True, stop=True)
                # out = s1/K - (asum@cw)/K  (cwk pre-scaled by 1/K on host)
                nc.vector.scalar_tensor_tensor(
                    oall[:, s * CK + ci:s * CK + ci + 1], d["s1"][ci][:],
                    1.0 / K, fps[:], ALU.mult, ALU.subtract)

        def mids(s):
            for g in range(GRP[s]):
                mid(s, g)
                # second square half after the first group's activations
                for ci in st[s].pop("sq_pending", []) if g == 0 else []:
                    nc.scalar.activation(
                        st[s]["xsq"][ci][:, HALF:N],
                        st[s]["xbf"][ci][:, HALF:N], AF.Square)

        def backs(s):
            d = st[s]
            # S1[c] = sum_n x — tensor_scalar accum keeps the 4x DVE mode;
            # placed here so it runs on DVE while ACT computes this sample's e
            d["s1"] = [work.tile([128, 1], F32, tag=f"s1{ci}",
                                 name=f"s1{ci}") for ci in range(CK)]
            for ci in range(CK):
                dump = dpool.tile([128, N], BF16, tag="dump", name="dump")
                nc.vector.tensor_scalar(
                    dump[:], d["xbf"][ci][:], 1.0, 0.0, ALU.mult, ALU.add,
                    accum_out=d["s1"][ci][:])
            for g in range(GRP[s]):
                backg(s, g)
            backf(s)

        front(0)
        front(1)
        mids(0)
        front(2)
        mids(1)
        backs(0)
        front(3)
        mids(2)
        backs(1)
        mids(3)
        backs(2)
        nc.sync.dma_start(out_d[:, 0:3 * CK], oall[:, 0:3 * CK])
        backs(3)
        nc.sync.dma_start(out_d[:, 3 * CK:BPC * CK], oall[:, 3 * CK:BPC * CK])
    nc.compile()

    # The greedy table-load inserter alternates natural_log <-> exp_and_others;
    # one load of natural_log_exp_and_others serves every Ln/Exp/Square here.
    for blk in nc.m.functions[0].blocks:
        insns = blk.instructions
        idxs = [i for i, it in enumerate(insns)
                if isinstance(it, mybir.InstLoadActFuncSet)]
        if not idxs:
            continue
        insns[idxs[0]].act_func_set_id = ACT_TABLE_LN_EXP
        for i in reversed(idxs[1:]):
            del insns[i]
    return nc


_NC = None
_NC_NPOS = None


def _get_nc(npos=None):
    global _NC, _NC_NPOS
    want = DEFAULT_NPOS if npos is None else npos
    if _NC is None or _NC_NPOS != want:
        _NC = build_nc(want)
        _NC_NPOS = want
    return _NC


def kernel(x, codewords, scale):
    x = np.ascontiguousarray(np.asarray(x, dtype=np.float32)).reshape(B, C, N)
    cw = np.asarray(codewords, dtype=np.float32)
    sc = np.asarray(scale, dtype=np.float32)

    # reorder codewords so positive scales come first (sign-split Exp)
    order = np.argsort(sc <= 0, kind="stable")
    cwo = cw[order].astype(np.float64)
    sco = sc[order].astype(np.float64)
    npos = int((sco > 0).sum())
    s2 = sco ** 2

    rx = (-2.0 * cwo.T * s2[None, :]).astype(ml_dtypes.bfloat16)
    rx = np.ascontiguousarray(rx.reshape(CK, 128, K))
    rq = np.tile(s2[None, :], (128, 1)).astype(ml_dtypes.bfloat16)
    c2s2 = (cwo ** 2).sum(axis=1) * s2
    hi = c2s2.astype(ml_dtypes.bfloat16)
    lo = (c2s2 - hi.astype(np.float64)).astype(ml_dtypes.bfloat16)
    crow = np.zeros((128, K), dtype=ml_dtypes.bfloat16)
    crow[0] = hi
    crow[1] = lo
    ones = np.ones((128, 128), dtype=ml_dtypes.bfloat16)
    cwk = (cwo / K).astype(np.float32)

    in_maps = []
    for core in range(NCORES):
        in_maps.append({
            "x": x[core * BPC:(core + 1) * BPC],
            "rx": rx, "rq": rq, "crow": crow, "ones": ones, "cwk": cwk,
        })

    res = run_bass_kernel_spmd(_get_nc(npos), in_maps,
                               core_ids=list(range(NCORES)))
    out = np.empty((B, C), dtype=np.float32)
    for core in range(NCORES):
        o = res.results[core]["out"]                    # [128, BPC*CK]
        for s in range(BPC):
            for ci in range(CK):
                out[core * BPC + s, 128 * ci:128 * (ci + 1)] = o[:, s * CK + ci]
    return out
